# revision 2
# baseline (speedup 1.0000x reference)
"""Trainium2 Bass kernel for nn_ColonyCBF (gnn_message_passing).

Computation (per row b of B=2^21):
    x_flat = concat(x_local[b], x_all[b, 1:7, :])            # 28 features
    h1 = relu(x_flat @ W1 + b1)                              # 64
    h2 = relu(h1 @ W2 + b2)                                  # 32
    out = 0.3 - softmax(|rw|) . x_local[b] + 0.1*(h2 @ W3 + b3)

Strategy: pure data-parallel over 8 NeuronCores.  On the host the batch is
packed into a transposed, 4-way "pack" layout (feature-on-partition) in bf16:

  xt [128, BC/4] bf16: partition strip 32r+f = feature f of batch quarter r;
     4 batch rows stream per PE column.

Device loop per 512-column chunk (2048 batch rows), full-array matmuls:
  L1:   2 matmuls, block-diag W1 stationary [128,128] -> pA (strips 0,1),
        pB (strips 2,3)
  risk: 1 matmul, [128,4] stationary (-softmax(|rw|) per strip diag) over
        xtile -> pG[0:4] (start=True)
  relu1 (ACT for pA, DVE for pB, per-partition bias) -> bf16 h1a/h1b
  L2:   2 matmuls, 2-copy block-diag W2 [128,64] -> pE rows 0:64 / 64:128
  relu2 (DVE tensor_scalar add+max) -> bf16 h2t
  L3:   1 matmul, [128,4] stationary (0.1*W3 per strip diag) over h2t,
        accumulated onto pG (start=False)
  final (ACT identity, bias 0.3+0.1*b3) [4,N] -> staging
  store: one [4, SGROUP*N] DMA per 16-chunk stage -> y[4, cols]

The L3 + final of chunk j are emitted in the middle of chunk j+1's matmul
stream (software pipelining) so the in-order PE never waits on relu2, and
input DMAs are prefetched one 16-chunk group ahead.
"""

import sys
import numpy as np
import ml_dtypes

sys.path.insert(0, "/opt/trn_rl_repo")

BF16 = ml_dtypes.bfloat16

B = 2_097_152
N_CORES = 8
BC = B // N_CORES            # rows per core
QUARTER = BC // 4            # columns of the packed layout
N = 512                      # columns (batch rows / 4) per chunk
N_CHUNKS = QUARTER // N      # 128
XGROUP = 16                  # chunks per input DMA (prefetched one group ahead)
SGROUP = 16                  # chunks per staging tile / store group

_BUILD_CACHE = {}


def _build(repeat=1):
    key = repeat
    if key in _BUILD_CACHE:
        return _BUILD_CACHE[key]
    import concourse.mybir as mybir
    import concourse.tile as tile
    from concourse import bacc
    from concourse.alu_op_type import AluOpType
    from contextlib import ExitStack

    dt = mybir.dt
    AF = mybir.ActivationFunctionType

    nc = bacc.Bacc("TRN2", target_bir_lowering=False, debug=False,
                   num_devices=N_CORES)
    xt_d = nc.dram_tensor("xt", [128, QUARTER], dt.bfloat16, kind="ExternalInput").ap()
    w1a_d = nc.dram_tensor("w1a", [128, 128], dt.bfloat16, kind="ExternalInput").ap()
    w1b_d = nc.dram_tensor("w1b", [128, 128], dt.bfloat16, kind="ExternalInput").ap()
    w2_d = nc.dram_tensor("w2", [128, 64], dt.bfloat16, kind="ExternalInput").ap()
    w3_d = nc.dram_tensor("w3", [128, 4], dt.bfloat16, kind="ExternalInput").ap()
    wr_d = nc.dram_tensor("wr", [128, 4], dt.bfloat16, kind="ExternalInput").ap()
    b1r_d = nc.dram_tensor("b1r", [128, 1], dt.float32, kind="ExternalInput").ap()
    b2r_d = nc.dram_tensor("b2r", [128, 1], dt.float32, kind="ExternalInput").ap()
    bf_d = nc.dram_tensor("bfin", [4, 1], dt.float32, kind="ExternalInput").ap()
    y_d = nc.dram_tensor("y", [4, QUARTER], dt.float32, kind="ExternalOutput").ap()

    with tile.TileContext(nc) as tc, ExitStack() as ctx:
        consts = ctx.enter_context(tc.tile_pool(name="consts", bufs=1))
        xpool = ctx.enter_context(tc.tile_pool(name="x", bufs=3))
        h1pool = ctx.enter_context(tc.tile_pool(name="h1", bufs=4))
        h2pool = ctx.enter_context(tc.tile_pool(name="h2", bufs=3))
        stpool = ctx.enter_context(tc.tile_pool(name="stage", bufs=2))
        psA = ctx.enter_context(tc.tile_pool(name="psA", bufs=2, space="PSUM"))
        psB = ctx.enter_context(tc.tile_pool(name="psB", bufs=2, space="PSUM"))
        psE = ctx.enter_context(tc.tile_pool(name="psE", bufs=2, space="PSUM"))
        psG = ctx.enter_context(tc.tile_pool(name="psG", bufs=2, space="PSUM"))

        def cl(dram, shape, dtype):
            t = consts.tile(shape, dtype, tag=dram.tensor.name)
            nc.sync.dma_start(out=t, in_=dram)
            return t

        s_w1a = cl(w1a_d, [128, 128], dt.bfloat16)
        s_w1b = cl(w1b_d, [128, 128], dt.bfloat16)
        s_w2 = cl(w2_d, [128, 64], dt.bfloat16)
        s_w3 = cl(w3_d, [128, 4], dt.bfloat16)
        s_wr = cl(wr_d, [128, 4], dt.bfloat16)
        s_b1r = cl(b1r_d, [128, 1], dt.float32)
        s_b2r = cl(b2r_d, [128, 1], dt.float32)
        s_bf = cl(bf_d, [4, 1], dt.float32)

        N_XG = N_CHUNKS // XGROUP

        def body():
            state = {"prev": None, "xbigs": {}, "stages": {}}

            def prefetch(g):
                if g >= N_XG or g in state["xbigs"]:
                    return
                xb = xpool.tile([128, XGROUP * N], dt.bfloat16,
                                name="xbig", tag="xbig")
                nc.sync.dma_start(out=xb,
                                  in_=xt_d[:, g * XGROUP * N:(g + 1) * XGROUP * N])
                state["xbigs"][g] = xb

            def flush_prev():
                """Emit L3 + final for the pipelined previous chunk."""
                prev = state["prev"]
                if prev is None:
                    return
                pG, h2t, j = prev
                nc.tensor.matmul(pG, s_w3, h2t, start=False, stop=True,
                                 skip_group_check=True)
                s, jq = j // SGROUP, j % SGROUP
                stage = state["stages"][s]
                nc.scalar.activation(stage[0:4, jq * N:(jq + 1) * N], pG,
                                     AF.Identity, bias=s_bf, scale=1.0)
                if jq == SGROUP - 1:
                    nc.sync.dma_start(
                        out=y_d[:, s * SGROUP * N:(s + 1) * SGROUP * N],
                        in_=stage)
                    del state["stages"][s]
                state["prev"] = None

            for j in range(N_CHUNKS):
                s, jq = j // SGROUP, j % SGROUP
                if jq == 0:
                    state["stages"][s] = stpool.tile([4, SGROUP * N],
                                                     dt.float32, name="stage",
                                                     tag="stage")
                g = j // XGROUP
                if j == 0:
                    prefetch(0)
                if j % XGROUP == 0:
                    prefetch(g + 1)
                xtile = state["xbigs"][g][:, (j % XGROUP) * N:(j % XGROUP + 1) * N]
                if j % XGROUP == XGROUP - 1:
                    state["xbigs"].pop(g - 1, None)
                pA = psA.tile([128, N], dt.float32)
                pB = psB.tile([128, N], dt.float32)
                pG = psG.tile([4, N], dt.float32, name="pG", tag="pG")
                # risk baseline into pG (accumulation group opened here,
                # closed by the pipelined L3 below)
                nc.tensor.matmul(pG, s_wr, xtile, start=True, stop=False,
                                 skip_group_check=True)
                # L1: two full-array matmuls with block-diag W1
                nc.tensor.matmul(pA, s_w1a, xtile, start=True, stop=True)
                # pipelined tail of the previous chunk sits here so the
                # in-order PE never waits for this chunk's relu1/relu2
                flush_prev()
                nc.tensor.matmul(pB, s_w1b, xtile, start=True, stop=True)
                h1a = h1pool.tile([128, N], dt.bfloat16, tag="h1")
                nc.scalar.activation(h1a, pA, AF.Relu, bias=s_b1r, scale=1.0)
                h1b = h1pool.tile([128, N], dt.bfloat16, tag="h1")
                nc.vector.tensor_scalar(out=h1b, in0=pB, scalar1=s_b1r,
                                        scalar2=0.0, op0=AluOpType.add,
                                        op1=AluOpType.max)
                pE = psE.tile([128, N], dt.float32)
                nc.tensor.matmul(pE[0:64, :], s_w2, h1a, start=True, stop=True,
                                 tile_position=(0, 0))
                nc.tensor.matmul(pE[64:128, :], s_w2, h1b, start=True, stop=True,
                                 tile_position=(0, 64))
                h2t = h2pool.tile([128, N], dt.bfloat16)
                nc.vector.tensor_scalar(out=h2t, in0=pE, scalar1=s_b2r,
                                        scalar2=0.0, op0=AluOpType.add,
                                        op1=AluOpType.max)
                state["prev"] = (pG, h2t, j)
            flush_prev()

        if repeat > 1:
            with tc.For_i(0, repeat, 1):
                body()
        else:
            body()

    nc.compile()
    _BUILD_CACHE[key] = nc
    return nc


def _prep_inputs(x_local, x_all, W1, b1, W2, b2, W3, b3, risk_weights):
    xf = np.empty((B, 28), np.float32)
    xf[:, :4] = x_local
    xf[:, 4:] = x_all[:, 1:7, :].reshape(B, 24)
    xb = xf.astype(BF16)
    X = xb.reshape(N_CORES, 4, QUARTER, 28)

    w1a = np.zeros((128, 128), BF16)
    w1a[0:28, 0:64] = W1
    w1a[32:60, 64:128] = W1
    w1b = np.zeros((128, 128), BF16)
    w1b[64:92, 0:64] = W1
    w1b[96:124, 64:128] = W1
    w2m = np.zeros((128, 64), BF16)
    w2m[0:64, 0:32] = W2
    w2m[64:128, 32:64] = W2
    w3m = np.zeros((128, 4), BF16)
    for r in range(4):
        w3m[32 * r:32 * r + 32, r] = 0.1 * W3[:, 0]
    a = np.abs(risk_weights.astype(np.float32))
    e = np.exp(a - a.max())
    wsm = e / e.sum()
    wrm = np.zeros((128, 4), BF16)
    for r in range(4):
        wrm[32 * r:32 * r + 4, r] = -wsm
    b1r = np.tile(np.asarray(b1, np.float32), 2).reshape(128, 1)
    b2r = np.tile(np.asarray(b2, np.float32), 4).reshape(128, 1)
    bfin = np.full((4, 1), 0.3 + 0.1 * float(b3[0]), np.float32)

    consts = dict(w1a=w1a, w1b=w1b, w2=w2m, w3=w3m, wr=wrm,
                  b1r=b1r, b2r=b2r, bfin=bfin)
    in_maps = []
    for c in range(N_CORES):
        xt = np.zeros((4, 32, QUARTER), BF16)
        # strip r holds batch quarter r
        xt[:, :28, :] = X[c].transpose(0, 2, 1)
        in_maps.append(dict(xt=xt.reshape(128, QUARTER), **consts))
    return in_maps


def run(in_maps, repeat=1):
    from concourse.bass_utils import run_bass_kernel_spmd
    nc = _build(repeat)
    return run_bass_kernel_spmd(nc, in_maps, core_ids=list(range(N_CORES)))


def kernel(x_local, x_all, W1, b1, W2, b2, W3, b3, risk_weights):
    x_local = np.asarray(x_local)
    x_all = np.asarray(x_all)
    in_maps = _prep_inputs(x_local, x_all, np.asarray(W1), np.asarray(b1),
                           np.asarray(W2), np.asarray(b2), np.asarray(W3),
                           np.asarray(b3), np.asarray(risk_weights))
    res = run(in_maps)
    out = np.empty(B, np.float32)
    for c in range(N_CORES):
        out[c * BC:(c + 1) * BC] = np.asarray(res.results[c]["y"],
                                              np.float32).reshape(-1)
    return out
